# revision 24
# baseline (speedup 1.0000x reference)
"""Trainium2 Bass kernel for a dense multi-head attention layer.

Problem (hardcoded shapes):
    hidden_states [2, 2048, 2048] fp32, attention_mask [2,1,1,2048] int32 (all ones),
    Wq/Wk/Wv/Wo [2048, 2048] fp32, biases [2048] fp32 (zeros in practice).
    out = MHA(hidden) with H=16 heads, head_dim=128.

Sharding: 8 cores = 2 batches x 4 head-groups (4 heads per core, tensor
parallel over heads). Each core computes q/k/v projections for its 4 heads,
attention, and a partial output projection; the host sums the 4 partials per
batch.

All matmul operands are bf16 (PSUM accumulation stays fp32), which runs at
full PE speed (1 cycle/row) and halves SBUF + DMA traffic vs fp32r. The
measured end-to-end relative error of the bf16 pipeline is ~6e-3.

Everything is SBUF-resident: h, the weights, and the q/k/v activations all
live on-chip for the whole kernel, so there is no DRAM roundtrip between the
projection phase and attention. The host supplies every tensor pre-arranged
into [128 partitions, ...] layout so each DMA moves >=4KB contiguous lines.

Engine budget per attention iteration (PE 10.2us): ACT does only the exp
stream (7.6us); the softmax denominator adds are split DVE/Pool; PSUM
evictions ride on DVE; DMA posts on SP.
"""
import os
import sys

if "/opt/trn_rl_repo" not in sys.path:
    sys.path.insert(0, "/opt/trn_rl_repo")

# If a previous run crashed the NEFF execution, a fresh NRT open with this
# flag recovers the cores instead of failing with EXEC_UNIT_UNRECOVERABLE.
os.environ.setdefault("NEURON_RT_RESET_CORES", "1")

import numpy as np

B, S, D, H, HD = 2, 2048, 2048, 16, 128
NCORES = 8
GROUPS = 4            # head-groups == cores per batch
GH = H // GROUPS      # heads per core = 4
GD = GH * HD          # 512 projection cols per core
ST = 512              # s/q/o tile width
NSB = S // 128        # 16 s-blocks
NEB = D // 128        # 16 e-blocks (contraction)
NST = S // ST         # 4 s-tiles
SCALE = 1.0 / float(np.sqrt(HD))

_RUNNER = None


def _build_nc():
    import concourse.tile as tile
    import concourse.bass_isa as bass_isa
    from concourse import bacc, mybir

    f32 = mybir.dt.float32
    bf16 = mybir.dt.bfloat16
    Exp = mybir.ActivationFunctionType.Exp

    nc = bacc.Bacc("TRN2", target_bir_lowering=False, debug=False,
                   num_devices=NCORES)

    # all inputs pre-arranged on host to [128, ...] partition-major bf16;
    # h is split st-major so the critical st=0 columns ship first with
    # contiguous lines
    hA = nc.dram_tensor("hA", [128, NEB, ST], bf16, kind="ExternalInput")
    hB = nc.dram_tensor("hB", [128, NST - 1, NEB, ST], bf16,
                        kind="ExternalInput")
    wqR = nc.dram_tensor("wqR", [128, NEB, GD], bf16, kind="ExternalInput")
    wkR = nc.dram_tensor("wkR", [128, NEB, GD], bf16, kind="ExternalInput")
    wvR = nc.dram_tensor("wvR", [128, NEB, GD], bf16, kind="ExternalInput")
    woR = nc.dram_tensor("woR", [128, GH, D], bf16, kind="ExternalInput")
    out = nc.dram_tensor("out", [S, D], f32, kind="ExternalOutput")

    with tile.TileContext(nc) as tc:
        with tc.tile_pool(name="qkv", bufs=1) as qkv_pool:
            # per-partition: 16KB each -> 64KB
            qt_all = qkv_pool.tile([128, GH, S], bf16)          # [hd, h, s]
            kt_all = qkv_pool.tile([128, GH, NSB, 128], bf16)   # [hd, h, kb, s]
            vt_all = qkv_pool.tile([128, GH, NSB, 128], bf16)   # [s, h, kb, hd]
            wo_sb = qkv_pool.tile([128, GH, D], bf16)

            # ---------------- phase 1: q/k/v projections ----------------
            with tc.tile_pool(name="w1", bufs=1) as w1, \
                 tc.tile_pool(name="hp", bufs=1) as hp, \
                 tc.tile_pool(name="ps1", bufs=8, space="PSUM") as ps1:
                wq_sb = w1.tile([128, NEB, GD], bf16)
                wk_sb = w1.tile([128, NEB, GD], bf16)
                wv_sb = w1.tile([128, NEB, GD], bf16)
                h_sbA = hp.tile([128, NEB, ST], bf16)
                h_sbB = hp.tile([128, NST - 1, NEB, ST], bf16)
                # DMA order = consumption order: st=0 sets at 2-eb
                # granularity (1MB/set, PE-paced), then wv (v-projection
                # needs it ~27us in), then hB by s-tile, then wo (used from
                # ~qt=1 of phase 2). Everything is >=2KB contiguous lines,
                # no single DMA >1MB.
                for ebg in range(NEB // 2):
                    e0, e1 = 2 * ebg, 2 * ebg + 2
                    nc.sync.dma_start(out=h_sbA[:, e0:e1, :],
                                      in_=hA[:, e0:e1, :])
                    nc.sync.dma_start(out=wq_sb[:, e0:e1, :],
                                      in_=wqR[:, e0:e1, :])
                    nc.sync.dma_start(out=wk_sb[:, e0:e1, :],
                                      in_=wkR[:, e0:e1, :])
                for ebg in range(4):
                    e0, e1 = 4 * ebg, 4 * ebg + 4
                    nc.sync.dma_start(out=wv_sb[:, e0:e1, :],
                                      in_=wvR[:, e0:e1, :])
                for st in range(1, NST):
                    for ebg in range(4):
                        e0, e1 = 4 * ebg, 4 * ebg + 4
                        nc.sync.dma_start(
                            out=h_sbB[:, st - 1, e0:e1, :],
                            in_=hB[:, st - 1, e0:e1, :])
                for cb in range(GH):
                    nc.sync.dma_start(out=wo_sb[:, cb, :], in_=woR[:, cb, :])

                def h_rhs(st, eb, lo, hi):
                    if st == 0:
                        return h_sbA[:, eb, lo:hi]
                    return h_sbB[:, st - 1, eb, lo:hi]

                for st in range(NST):
                    sl = slice(st * ST, (st + 1) * ST)
                    if st == 0:
                        # eb-outer over 8 live accumulators: consume input
                        # chunks in arrival order so the PE tracks the DMA
                        # stream instead of stalling per accumulation.
                        pss = {}
                        for h in range(GH):
                            for t in range(2):
                                pss[(h, t)] = ps1.tile([128, ST], f32,
                                                       tag="ps1",
                                                       name=f"psqk{h}{t}")
                        for eb in range(NEB):
                            for h in range(GH):
                                for t, w_sb in ((0, wq_sb), (1, wk_sb)):
                                    nc.tensor.matmul(
                                        pss[(h, t)],
                                        w_sb[:, eb, h * HD:(h + 1) * HD],
                                        h_rhs(st, eb, 0, ST),
                                        start=(eb == 0), stop=(eb == NEB - 1))
                        for h in range(GH):
                            # split across ACT and DVE so the 8 evictions
                            # drain in parallel and free PSUM for the v path
                            nc.scalar.copy(qt_all[:, h, sl], pss[(h, 0)])
                            nc.vector.tensor_copy(
                                kt_all[:, h, st * 4:(st + 1) * 4, :],
                                pss[(h, 1)])
                    else:
                        for h in range(GH):
                            for t, w_sb in ((0, wq_sb), (1, wk_sb)):
                                ps = ps1.tile([128, ST], f32, tag="ps1")
                                for eb in range(NEB):
                                    nc.tensor.matmul(
                                        ps, w_sb[:, eb, h * HD:(h + 1) * HD],
                                        h_rhs(st, eb, 0, ST),
                                        start=(eb == 0), stop=(eb == NEB - 1))
                                if t == 0:
                                    nc.scalar.copy(qt_all[:, h, sl], ps)
                                else:
                                    nc.scalar.copy(
                                        kt_all[:, h, st * 4:(st + 1) * 4, :],
                                        ps)
                    for j in range(ST // 128):
                        ps = ps1.tile([128, GD], f32, tag="ps1")
                        for eb in range(NEB):
                            nc.tensor.matmul(
                                ps, h_rhs(st, eb, j * 128, (j + 1) * 128),
                                wv_sb[:, eb, :],
                                start=(eb == 0), stop=(eb == NEB - 1))
                        # cols of ps are heads-major: exactly vt_all[:, :, sb, :]
                        nc.vector.tensor_copy(vt_all[:, :, st * 4 + j, :], ps)

            # ------- phase 2+3: attention fused with output projection -------
            # qt outer / heads inner. The output projection for query tile
            # qt-1 is interleaved INTO the kb2 loop of tile qt, with each
            # proj matmul pair issued BEFORE the PV pair it precedes: the PE
            # queue is in-order, so this guarantees PE always has
            # exp-independent work while ACT streams the exps (ACT needs
            # ~1.0us per kb2 pair vs 0.85us of dependent PE work).
            with tc.tile_pool(name="expp", bufs=4) as expp, \
                 tc.tile_pool(name="sm", bufs=2) as sm, \
                 tc.tile_pool(name="attn2", bufs=2) as attn2, \
                 tc.tile_pool(name="ev3", bufs=4) as ev3, \
                 tc.tile_pool(name="ps_s", bufs=2, space="PSUM") as ps_s, \
                 tc.tile_pool(name="pap", bufs=2, space="PSUM") as pap, \
                 tc.tile_pool(name="pop", bufs=2, space="PSUM") as pop:

                class ProjState:
                    """Issues one ot-column pair of the (qt_prev, j) output
                    projection per call; 8 calls complete the s-block."""

                    def __init__(self, prev, qt_prev, j):
                        self.prev = prev
                        self.j = j
                        self.sb = qt_prev * (ST // 128) + j
                        self.ov = ev3.tile([128, D], f32, tag="ov",
                                           name=f"ov{self.sb}")
                        self.po = None

                    def step(self, kb2):
                        ot, half = kb2 // 2, kb2 % 2
                        if half == 0:
                            self.po = pop.tile([128, ST], f32, tag="po")
                        for cb in (0, 1) if half == 0 else (2, 3):
                            nc.tensor.matmul(
                                self.po,
                                self.prev[cb][:, self.j * 128:
                                              (self.j + 1) * 128],
                                wo_sb[:, cb, ot * ST:(ot + 1) * ST],
                                start=(cb == 0), stop=(cb == GH - 1))
                        if half == 1:
                            nc.vector.tensor_copy(
                                self.ov[:, ot * ST:(ot + 1) * ST], self.po)
                            if ot % 2 == 1:
                                # ship each half as soon as its columns are
                                # written: halves the DMA release latency and
                                # doubles engine parallelism per block
                                r0 = self.sb * 128
                                c0, c1 = (ot - 1) * ST, (ot + 1) * ST
                                nc.sync.dma_start(
                                    out=out[r0:r0 + 128, c0:c1],
                                    in_=self.ov[:, c0:c1])

                prev_attn = None
                for qt in range(NST):
                    attn_t = []
                    for h in range(GH):
                        proj = (ProjState(prev_attn, qt - 1, h)
                                if prev_attn is not None else None)
                        # at h=0 the proj source at(qt-1, h3) is only ~1us
                        # old; delay its proj steps four kb2 slots so the PE
                        # queue never blocks on it
                        delay = 4 if h == 0 else 0
                        qs = qt_all[:, h, qt * ST:(qt + 1) * ST]
                        expt_halves = [
                            expp.tile([128, NSB // 2, ST], bf16, tag="expt",
                                      name=f"expt{h}{half}")
                            for half in range(2)]
                        esd = sm.tile([128, ST], f32, tag="esd")
                        esp = sm.tile([128, ST], f32, tag="esp")
                        pa = pap.tile([128, ST], f32, tag="pa")

                        for kb2 in range(NSB // 2):
                            expt = expt_halves[kb2 // 4]
                            kbo = (kb2 % 4) * 2
                            ps = ps_s.tile([128, 2, ST], f32, tag="ps")
                            for half in range(2):
                                kb = kb2 * 2 + half
                                nc.tensor.matmul(ps[:, half, :],
                                                 kt_all[:, h, kb, :], qs,
                                                 start=True, stop=True)
                            pair = expt[:, kbo:kbo + 2, :]
                            nc.scalar.activation(pair, ps, Exp, scale=SCALE)
                            # denominator adds: first half on DVE, second on
                            # Pool, so neither engine blocks the exp->PV chain
                            eng = nc.vector if kb2 < 4 else nc.gpsimd
                            dst = esd if kb2 < 4 else esp
                            if kb2 % 4 == 0:
                                eng.tensor_add(dst, expt[:, kbo, :],
                                               expt[:, kbo + 1, :])
                            else:
                                eng.tensor_add(dst, dst, expt[:, kbo, :])
                                eng.tensor_add(dst, dst, expt[:, kbo + 1, :])
                            if proj is not None and kb2 >= delay:
                                proj.step(kb2 - delay)
                            # PV for the PREVIOUS kb2: one slot of extra
                            # latency slack on the exp->PV edge
                            if kb2 > 0:
                                pkbo = ((kb2 - 1) % 4) * 2
                                pexpt = expt_halves[(kb2 - 1) // 4]
                                for half in range(2):
                                    kb = (kb2 - 1) * 2 + half
                                    nc.tensor.matmul(pa,
                                                     vt_all[:, h, kb, :],
                                                     pexpt[:, pkbo + half, :],
                                                     start=(kb == 0),
                                                     stop=False)
                        lexpt = expt_halves[1]
                        lkbo = ((NSB // 2 - 1) % 4) * 2
                        for half in range(2):
                            kb = (NSB // 2 - 1) * 2 + half
                            nc.tensor.matmul(pa, vt_all[:, h, kb, :],
                                             lexpt[:, lkbo + half, :],
                                             start=False,
                                             stop=(kb == NSB - 1))
                        if proj is not None:
                            for step in range(NSB // 2 - delay, NSB // 2):
                                proj.step(step)
                        nc.vector.tensor_add(esd, esd, esp)
                        bcsum = sm.tile([128, ST], f32, tag="bcsum")
                        nc.gpsimd.partition_all_reduce(
                            bcsum, esd, 128, bass_isa.ReduceOp.add)
                        brc = sm.tile([128, ST], f32, tag="brc")
                        nc.vector.reciprocal(brc, bcsum)
                        at = attn2.tile([128, ST], bf16, name=f"at{h}",
                                        tag=f"at{h}")
                        nc.vector.tensor_mul(at, pa, brc)
                        attn_t.append(at)
                    prev_attn = attn_t
                for j in range(ST // 128):
                    proj = ProjState(prev_attn, NST - 1, j)
                    for kb2 in range(NSB // 2):
                        proj.step(kb2)

    nc.compile()
    return nc


def _get_runner():
    global _RUNNER
    if _RUNNER is None:
        _RUNNER = _build_nc()
    return _RUNNER


def _to_bf16(x: np.ndarray):
    import ml_dtypes
    return np.ascontiguousarray(x, dtype=np.float32).astype(ml_dtypes.bfloat16)


def _part_major(mat: np.ndarray, nb: int) -> np.ndarray:
    """[nb*128, cols] -> [128, nb, cols] partition-major layout."""
    rows, cols = mat.shape
    return np.ascontiguousarray(
        mat.reshape(nb, 128, cols).transpose(1, 0, 2))


def _prepare_in_maps(hidden_states, Wq, Wk, Wv, Wo):
    hidden = np.asarray(hidden_states, dtype=np.float32)
    wq = np.asarray(Wq, dtype=np.float32)
    wk = np.asarray(Wk, dtype=np.float32)
    wv = np.asarray(Wv, dtype=np.float32)
    wo = np.asarray(Wo, dtype=np.float32)
    hA, hB = [], []
    for b in range(B):
        hpm = _part_major(np.ascontiguousarray(hidden[b].T), NEB)
        hA.append(_to_bf16(hpm[:, :, 0:ST]))
        # [128, NEB, (NST-1)*ST] -> [128, NST-1, NEB, ST] st-major
        rest = hpm[:, :, ST:].reshape(128, NEB, NST - 1, ST)
        hB.append(_to_bf16(rest.transpose(0, 2, 1, 3)))
    in_maps = []
    for core in range(NCORES):
        b, g = divmod(core, GROUPS)
        rows = slice(g * GD, (g + 1) * GD)
        in_maps.append({
            "hA": hA[b],
            "hB": hB[b],
            "wqR": _to_bf16(_part_major(
                np.ascontiguousarray(wq[rows, :].T), NEB)),
            "wkR": _to_bf16(_part_major(
                np.ascontiguousarray(wk[rows, :].T), NEB)),
            "wvR": _to_bf16(_part_major(
                np.ascontiguousarray(wv[rows, :].T), NEB)),
            "woR": _to_bf16(_part_major(
                np.ascontiguousarray(wo[:, rows].T), GH)),
        })
    return in_maps


def _run_device_trace(in_maps, tmpdir=None):
    from concourse.bass_utils import run_bass_kernel_spmd
    nc = _get_runner()
    return run_bass_kernel_spmd(nc, in_maps, core_ids=list(range(NCORES)),
                                trace=True, tmpdir=tmpdir)


def _run_device(in_maps, trace=False):
    from concourse.bass_utils import run_bass_kernel_spmd
    nc = _get_runner()
    try:
        return run_bass_kernel_spmd(nc, in_maps, core_ids=list(range(NCORES)),
                                    trace=trace)
    except Exception:
        # Transient device failures (rare) are recoverable by reopening the
        # backend with NEURON_RT_RESET_CORES=1. Retry once.
        try:
            import jax
            jax.clear_caches()
            try:
                jax.extend.backend.clear_backends()
            except Exception:
                jax._src.api.clear_backends()
        except Exception:
            pass
        return run_bass_kernel_spmd(nc, in_maps, core_ids=list(range(NCORES)),
                                    trace=trace)


def _numpy_reference(hidden_states, attention_mask, Wq, bq, Wk, bk, Wv, bv,
                     Wo, bo):
    """Exact fallback for inputs the fast path does not handle."""
    h = np.asarray(hidden_states, dtype=np.float32)
    mask = np.asarray(attention_mask)
    q = h @ np.asarray(Wq, np.float32).T + np.asarray(bq, np.float32)
    k = h @ np.asarray(Wk, np.float32).T + np.asarray(bk, np.float32)
    v = h @ np.asarray(Wv, np.float32).T + np.asarray(bv, np.float32)
    q = q.reshape(B, S, H, HD).transpose(0, 2, 1, 3)
    k = k.reshape(B, S, H, HD).transpose(0, 2, 1, 3)
    v = v.reshape(B, S, H, HD).transpose(0, 2, 1, 3)
    scores = (q @ k.transpose(0, 1, 3, 2)).astype(np.float32) * SCALE
    scores = np.where(mask == 0, np.float32(-1e9), scores)
    scores -= scores.max(axis=-1, keepdims=True)
    probs = np.exp(scores, dtype=np.float32)
    probs /= probs.sum(axis=-1, keepdims=True)
    attn = probs @ v
    attn = attn.transpose(0, 2, 1, 3).reshape(B, S, D)
    out = attn @ np.asarray(Wo, np.float32).T + np.asarray(bo, np.float32)
    return out.astype(np.float32)


def kernel(hidden_states, attention_mask, Wq, bq, Wk, bk, Wv, bv, Wo, bo):
    mask = np.asarray(attention_mask)
    bq_np = np.asarray(bq, dtype=np.float32)
    if (mask == 0).any() or np.any(bq_np):
        # general (never hit with the reference setup_inputs): bq shifts
        # scores per-key and a masked key changes the softmax support —
        # neither is representable in the fast path's fused layout.
        return _numpy_reference(hidden_states, attention_mask, Wq, bq, Wk,
                                bk, Wv, bv, Wo, bo)

    in_maps = _prepare_in_maps(hidden_states, Wq, Wk, Wv, Wo)
    res = _run_device(in_maps)

    # bk only adds a per-query constant to scores (softmax-invariant).
    # bv passes through the probs (rows sum to 1): out += bv @ Wo.T. bo adds.
    extra = (np.asarray(bv, np.float64) @ np.asarray(Wo, np.float64).T
             + np.asarray(bo, np.float64))
    out = np.empty((B, S, D), dtype=np.float32)
    for b in range(B):
        acc = np.zeros((S, D), dtype=np.float64)
        for g in range(GROUPS):
            acc += res.results[b * GROUPS + g]["out"]
        out[b] = (acc + extra).astype(np.float32)
    return out


# revision 31
# speedup vs baseline: 1.0783x; 1.0783x over previous
"""Trainium2 Bass kernel for a dense multi-head attention layer.

Problem (hardcoded shapes):
    hidden_states [2, 2048, 2048] fp32, attention_mask [2,1,1,2048] int32 (all ones),
    Wq/Wk/Wv/Wo [2048, 2048] fp32, biases [2048] fp32 (zeros in practice).
    out = MHA(hidden) with H=16 heads, head_dim=128.

Sharding: 8 cores = 2 batches x 4 head-groups (4 heads per core, tensor
parallel over heads). Each core computes q/k/v projections for its 4 heads,
attention, and a partial output projection; the host sums the 4 partials per
batch.

All matmul operands are bf16 (PSUM accumulation stays fp32), which runs at
full PE speed (1 cycle/row) and halves SBUF + DMA traffic vs fp32r. The
measured end-to-end relative error of the bf16 pipeline is ~6e-3.

Everything is SBUF-resident: h, the weights, and the q/k/v activations all
live on-chip for the whole kernel, so there is no DRAM roundtrip between the
projection phase and attention. The host supplies every tensor pre-arranged
into [128 partitions, ...] layout so each DMA moves >=4KB contiguous lines.

Engine budget per attention iteration (PE 10.2us): ACT does only the exp
stream (7.6us); the softmax denominator adds are split DVE/Pool; PSUM
evictions ride on DVE; DMA posts on SP.
"""
import os
import sys

if "/opt/trn_rl_repo" not in sys.path:
    sys.path.insert(0, "/opt/trn_rl_repo")

# If a previous run crashed the NEFF execution, a fresh NRT open with this
# flag recovers the cores instead of failing with EXEC_UNIT_UNRECOVERABLE.
os.environ.setdefault("NEURON_RT_RESET_CORES", "1")

import numpy as np

B, S, D, H, HD = 2, 2048, 2048, 16, 128
NCORES = 8
GROUPS = 4            # head-groups == cores per batch
GH = H // GROUPS      # heads per core = 4
GD = GH * HD          # 512 projection cols per core
ST = 512              # s/q/o tile width
NSB = S // 128        # 16 s-blocks
NEB = D // 128        # 16 e-blocks (contraction)
NST = S // ST         # 4 s-tiles
SCALE = 1.0 / float(np.sqrt(HD))

_RUNNER = None


def _build_nc():
    import concourse.tile as tile
    import concourse.bass_isa as bass_isa
    from concourse import bacc, mybir

    f32 = mybir.dt.float32
    bf16 = mybir.dt.bfloat16
    Exp = mybir.ActivationFunctionType.Exp

    nc = bacc.Bacc("TRN2", target_bir_lowering=False, debug=False,
                   num_devices=NCORES)

    # all inputs pre-arranged on host to [128, ...] partition-major bf16;
    # h is split st-major so the critical st=0 columns ship first with
    # contiguous lines
    hA = nc.dram_tensor("hA", [128, NEB, ST], bf16, kind="ExternalInput")
    hB = nc.dram_tensor("hB", [128, NST - 1, NEB, ST], bf16,
                        kind="ExternalInput")
    wqR = nc.dram_tensor("wqR", [128, NEB, GD], bf16, kind="ExternalInput")
    wkR = nc.dram_tensor("wkR", [128, NEB, GD], bf16, kind="ExternalInput")
    wvR = nc.dram_tensor("wvR", [128, NEB, GD], bf16, kind="ExternalInput")
    woR = nc.dram_tensor("woR", [128, GH, D], bf16, kind="ExternalInput")
    out = nc.dram_tensor("out", [S, D], f32, kind="ExternalOutput")

    with tile.TileContext(nc) as tc:
        with tc.tile_pool(name="qkv", bufs=1) as qkv_pool:
            # per-partition: 16KB each -> 64KB
            qt_all = qkv_pool.tile([128, GH, S], bf16)          # [hd, h, s]
            kt_all = qkv_pool.tile([128, GH, NSB, 128], bf16)   # [hd, h, kb, s]
            vt_all = qkv_pool.tile([128, GH, NSB, 128], bf16)   # [s, h, kb, hd]
            wo_sb = qkv_pool.tile([128, GH, D], bf16)

            # ---------------- phase 1: q/k/v projections ----------------
            with tc.tile_pool(name="w1", bufs=1) as w1, \
                 tc.tile_pool(name="hp", bufs=1) as hp, \
                 tc.tile_pool(name="ps1", bufs=8, space="PSUM") as ps1:
                wq_sb = w1.tile([128, NEB, GD], bf16)
                wk_sb = w1.tile([128, NEB, GD], bf16)
                wv_sb = w1.tile([128, NEB, GD], bf16)
                h_sbA = hp.tile([128, NEB, ST], bf16)
                h_sbB = hp.tile([128, NST - 1, NEB, ST], bf16)
                # DMA order = consumption order: st=0 sets at 2-eb
                # granularity (1MB/set, PE-paced), then wv (v-projection
                # needs it ~27us in), then hB by s-tile, then wo (used from
                # ~qt=1 of phase 2). Everything is >=2KB contiguous lines,
                # no single DMA >1MB.
                ebgs = [(0, 1), (1, 2)] + [(2 * g, 2 * g + 2)
                                          for g in range(1, NEB // 2)]
                for e0, e1 in ebgs:
                    nc.sync.dma_start(out=h_sbA[:, e0:e1, :],
                                      in_=hA[:, e0:e1, :])
                    nc.sync.dma_start(out=wq_sb[:, e0:e1, :],
                                      in_=wqR[:, e0:e1, :])
                    nc.sync.dma_start(out=wk_sb[:, e0:e1, :],
                                      in_=wkR[:, e0:e1, :])
                for ebg in range(4):
                    e0, e1 = 4 * ebg, 4 * ebg + 4
                    nc.sync.dma_start(out=wv_sb[:, e0:e1, :],
                                      in_=wvR[:, e0:e1, :])
                for st in range(1, NST):
                    for ebg in range(4):
                        e0, e1 = 4 * ebg, 4 * ebg + 4
                        nc.sync.dma_start(
                            out=h_sbB[:, st - 1, e0:e1, :],
                            in_=hB[:, st - 1, e0:e1, :])
                for cb in range(GH):
                    nc.sync.dma_start(out=wo_sb[:, cb, :], in_=woR[:, cb, :])

                def h_rhs(st, eb, lo, hi):
                    if st == 0:
                        return h_sbA[:, eb, lo:hi]
                    return h_sbB[:, st - 1, eb, lo:hi]

                for st in range(NST):
                    sl = slice(st * ST, (st + 1) * ST)
                    if st == 0:
                        # eb-outer over 8 live accumulators: consume input
                        # chunks in arrival order so the PE tracks the DMA
                        # stream instead of stalling per accumulation.
                        pss = {}
                        for h in range(GH):
                            for t in range(2):
                                pss[(h, t)] = ps1.tile([128, ST], f32,
                                                       tag="ps1",
                                                       name=f"psqk{h}{t}")
                        for eb in range(NEB):
                            for h in range(GH):
                                for t, w_sb in ((0, wq_sb), (1, wk_sb)):
                                    nc.tensor.matmul(
                                        pss[(h, t)],
                                        w_sb[:, eb, h * HD:(h + 1) * HD],
                                        h_rhs(st, eb, 0, ST),
                                        start=(eb == 0), stop=(eb == NEB - 1))
                        for h in range(GH):
                            # split across ACT and DVE so the 8 evictions
                            # drain in parallel and free PSUM for the v path
                            nc.scalar.copy(qt_all[:, h, sl], pss[(h, 0)])
                            nc.vector.tensor_copy(
                                kt_all[:, h, st * 4:(st + 1) * 4, :],
                                pss[(h, 1)])
                    else:
                        for h in range(GH):
                            for t, w_sb in ((0, wq_sb), (1, wk_sb)):
                                ps = ps1.tile([128, ST], f32, tag="ps1")
                                for eb in range(NEB):
                                    nc.tensor.matmul(
                                        ps, w_sb[:, eb, h * HD:(h + 1) * HD],
                                        h_rhs(st, eb, 0, ST),
                                        start=(eb == 0), stop=(eb == NEB - 1))
                                if t == 0:
                                    nc.scalar.copy(qt_all[:, h, sl], ps)
                                else:
                                    nc.scalar.copy(
                                        kt_all[:, h, st * 4:(st + 1) * 4, :],
                                        ps)
                    for j in range(ST // 128):
                        ps = ps1.tile([128, GD], f32, tag="ps1")
                        for eb in range(NEB):
                            nc.tensor.matmul(
                                ps, h_rhs(st, eb, j * 128, (j + 1) * 128),
                                wv_sb[:, eb, :],
                                start=(eb == 0), stop=(eb == NEB - 1))
                        # cols of ps are heads-major: exactly vt_all[:, :, sb, :]
                        nc.vector.tensor_copy(vt_all[:, :, st * 4 + j, :], ps)

            # ------- phase 2+3: attention fused with output projection -------
            # qt outer / heads inner. The output projection for query tile
            # qt-1 is interleaved INTO the kb2 loop of tile qt, with each
            # proj matmul pair issued BEFORE the PV pair it precedes: the PE
            # queue is in-order, so this guarantees PE always has
            # exp-independent work while ACT streams the exps (ACT needs
            # ~1.0us per kb2 pair vs 0.85us of dependent PE work).
            with tc.tile_pool(name="expp", bufs=4) as expp, \
                 tc.tile_pool(name="sm", bufs=2) as sm, \
                 tc.tile_pool(name="attn2", bufs=2) as attn2, \
                 tc.tile_pool(name="ev3", bufs=4) as ev3, \
                 tc.tile_pool(name="ps_s", bufs=2, space="PSUM") as ps_s, \
                 tc.tile_pool(name="pap", bufs=2, space="PSUM") as pap, \
                 tc.tile_pool(name="pop", bufs=2, space="PSUM") as pop:

                class ProjState:
                    """Issues one ot-column pair of the (qt_prev, j) output
                    projection per call; 8 calls complete the s-block."""

                    def __init__(self, prev, qt_prev, j):
                        self.prev = prev
                        self.j = j
                        self.sb = qt_prev * (ST // 128) + j
                        self.ov = ev3.tile([128, D], f32, tag="ov",
                                           name=f"ov{self.sb}")
                        self.po = None

                    def step(self, kb2):
                        ot, half = kb2 // 2, kb2 % 2
                        if half == 0:
                            self.po = pop.tile([128, ST], f32, tag="po")
                        for cb in (0, 1) if half == 0 else (2, 3):
                            nc.tensor.matmul(
                                self.po,
                                self.prev[cb][:, self.j * 128:
                                              (self.j + 1) * 128],
                                wo_sb[:, cb, ot * ST:(ot + 1) * ST],
                                start=(cb == 0), stop=(cb == GH - 1))
                        if half == 1:
                            nc.vector.tensor_copy(
                                self.ov[:, ot * ST:(ot + 1) * ST], self.po)
                            if ot % 2 == 1:
                                # ship each half as soon as its columns are
                                # written: halves the DMA release latency and
                                # doubles engine parallelism per block
                                r0 = self.sb * 128
                                c0, c1 = (ot - 1) * ST, (ot + 1) * ST
                                nc.sync.dma_start(
                                    out=out[r0:r0 + 128, c0:c1],
                                    in_=self.ov[:, c0:c1])

                prev_attn = None
                for qt in range(NST):
                    attn_t = []
                    for h in range(GH):
                        proj = (ProjState(prev_attn, qt - 1, h)
                                if prev_attn is not None else None)
                        # at h=0 the proj source at(qt-1, h3) is only ~1us
                        # old; delay its proj steps four kb2 slots so the PE
                        # queue never blocks on it
                        delay = 4 if h == 0 else 0
                        qs = qt_all[:, h, qt * ST:(qt + 1) * ST]
                        expt_halves = [
                            expp.tile([128, NSB // 2, ST], bf16, tag="expt",
                                      name=f"expt{h}{half}")
                            for half in range(2)]
                        esd = sm.tile([128, ST], f32, tag="esd")
                        esp = sm.tile([128, ST], f32, tag="esp")
                        pa = pap.tile([128, ST], f32, tag="pa")

                        for kb2 in range(NSB // 2):
                            expt = expt_halves[kb2 // 4]
                            kbo = (kb2 % 4) * 2
                            ps = ps_s.tile([128, 2, ST], f32, tag="ps")
                            for half in range(2):
                                kb = kb2 * 2 + half
                                nc.tensor.matmul(ps[:, half, :],
                                                 kt_all[:, h, kb, :], qs,
                                                 start=True, stop=True)
                            pair = expt[:, kbo:kbo + 2, :]
                            nc.scalar.activation(pair, ps, Exp, scale=SCALE)
                            # denominator adds: first half on DVE, second on
                            # Pool, so neither engine blocks the exp->PV chain
                            eng = nc.vector if kb2 < 4 else nc.gpsimd
                            dst = esd if kb2 < 4 else esp
                            if kb2 % 4 == 0:
                                eng.tensor_add(dst, expt[:, kbo, :],
                                               expt[:, kbo + 1, :])
                            else:
                                eng.tensor_add(dst, dst, expt[:, kbo, :])
                                eng.tensor_add(dst, dst, expt[:, kbo + 1, :])
                            if proj is not None and kb2 >= delay:
                                proj.step(kb2 - delay)
                            # PV for the PREVIOUS kb2: one slot of extra
                            # latency slack on the exp->PV edge
                            if kb2 > 0:
                                pkbo = ((kb2 - 1) % 4) * 2
                                pexpt = expt_halves[(kb2 - 1) // 4]
                                for half in range(2):
                                    kb = (kb2 - 1) * 2 + half
                                    nc.tensor.matmul(pa,
                                                     vt_all[:, h, kb, :],
                                                     pexpt[:, pkbo + half, :],
                                                     start=(kb == 0),
                                                     stop=False)
                        lexpt = expt_halves[1]
                        lkbo = ((NSB // 2 - 1) % 4) * 2
                        for half in range(2):
                            kb = (NSB // 2 - 1) * 2 + half
                            nc.tensor.matmul(pa, vt_all[:, h, kb, :],
                                             lexpt[:, lkbo + half, :],
                                             start=False,
                                             stop=(kb == NSB - 1))
                        if proj is not None:
                            for step in range(NSB // 2 - delay, NSB // 2):
                                proj.step(step)
                        nc.vector.tensor_add(esd, esd, esp)
                        bcsum = sm.tile([128, ST], f32, tag="bcsum")
                        nc.gpsimd.partition_all_reduce(
                            bcsum, esd, 128, bass_isa.ReduceOp.add)
                        brc = sm.tile([128, ST], f32, tag="brc")
                        nc.vector.reciprocal(brc, bcsum)
                        at = attn2.tile([128, ST], bf16, name=f"at{h}",
                                        tag=f"at{h}")
                        nc.vector.tensor_mul(at, pa, brc)
                        attn_t.append(at)
                    prev_attn = attn_t
                for j in range(ST // 128):
                    proj = ProjState(prev_attn, NST - 1, j)
                    for kb2 in range(NSB // 2):
                        proj.step(kb2)

    nc.compile()
    return nc


def _get_runner():
    global _RUNNER
    if _RUNNER is None:
        _RUNNER = _build_nc()
    return _RUNNER


def _to_bf16(x: np.ndarray):
    import ml_dtypes
    return np.ascontiguousarray(x, dtype=np.float32).astype(ml_dtypes.bfloat16)


def _part_major(mat: np.ndarray, nb: int) -> np.ndarray:
    """[nb*128, cols] -> [128, nb, cols] partition-major layout."""
    rows, cols = mat.shape
    return np.ascontiguousarray(
        mat.reshape(nb, 128, cols).transpose(1, 0, 2))


def _prepare_in_maps(hidden_states, Wq, Wk, Wv, Wo):
    hidden = np.asarray(hidden_states, dtype=np.float32)
    wq = np.asarray(Wq, dtype=np.float32)
    wk = np.asarray(Wk, dtype=np.float32)
    wv = np.asarray(Wv, dtype=np.float32)
    wo = np.asarray(Wo, dtype=np.float32)
    hA, hB = [], []
    for b in range(B):
        hpm = _part_major(np.ascontiguousarray(hidden[b].T), NEB)
        hA.append(_to_bf16(hpm[:, :, 0:ST]))
        # [128, NEB, (NST-1)*ST] -> [128, NST-1, NEB, ST] st-major
        rest = hpm[:, :, ST:].reshape(128, NEB, NST - 1, ST)
        hB.append(_to_bf16(rest.transpose(0, 2, 1, 3)))
    in_maps = []
    for core in range(NCORES):
        b, g = divmod(core, GROUPS)
        rows = slice(g * GD, (g + 1) * GD)
        in_maps.append({
            "hA": hA[b],
            "hB": hB[b],
            "wqR": _to_bf16(_part_major(
                np.ascontiguousarray(wq[rows, :].T), NEB)),
            "wkR": _to_bf16(_part_major(
                np.ascontiguousarray(wk[rows, :].T), NEB)),
            "wvR": _to_bf16(_part_major(
                np.ascontiguousarray(wv[rows, :].T), NEB)),
            "woR": _to_bf16(_part_major(
                np.ascontiguousarray(wo[:, rows].T), GH)),
        })
    return in_maps


def _run_device_trace(in_maps, tmpdir=None):
    from concourse.bass_utils import run_bass_kernel_spmd
    nc = _get_runner()
    return run_bass_kernel_spmd(nc, in_maps, core_ids=list(range(NCORES)),
                                trace=True, tmpdir=tmpdir)


def _run_device(in_maps, trace=False):
    from concourse.bass_utils import run_bass_kernel_spmd
    nc = _get_runner()
    try:
        return run_bass_kernel_spmd(nc, in_maps, core_ids=list(range(NCORES)),
                                    trace=trace)
    except Exception:
        # Transient device failures (rare) are recoverable by reopening the
        # backend with NEURON_RT_RESET_CORES=1. Retry once.
        try:
            import jax
            jax.clear_caches()
            try:
                jax.extend.backend.clear_backends()
            except Exception:
                jax._src.api.clear_backends()
        except Exception:
            pass
        return run_bass_kernel_spmd(nc, in_maps, core_ids=list(range(NCORES)),
                                    trace=trace)


def _numpy_reference(hidden_states, attention_mask, Wq, bq, Wk, bk, Wv, bv,
                     Wo, bo):
    """Exact fallback for inputs the fast path does not handle."""
    h = np.asarray(hidden_states, dtype=np.float32)
    mask = np.asarray(attention_mask)
    q = h @ np.asarray(Wq, np.float32).T + np.asarray(bq, np.float32)
    k = h @ np.asarray(Wk, np.float32).T + np.asarray(bk, np.float32)
    v = h @ np.asarray(Wv, np.float32).T + np.asarray(bv, np.float32)
    q = q.reshape(B, S, H, HD).transpose(0, 2, 1, 3)
    k = k.reshape(B, S, H, HD).transpose(0, 2, 1, 3)
    v = v.reshape(B, S, H, HD).transpose(0, 2, 1, 3)
    scores = (q @ k.transpose(0, 1, 3, 2)).astype(np.float32) * SCALE
    scores = np.where(mask == 0, np.float32(-1e9), scores)
    scores -= scores.max(axis=-1, keepdims=True)
    probs = np.exp(scores, dtype=np.float32)
    probs /= probs.sum(axis=-1, keepdims=True)
    attn = probs @ v
    attn = attn.transpose(0, 2, 1, 3).reshape(B, S, D)
    out = attn @ np.asarray(Wo, np.float32).T + np.asarray(bo, np.float32)
    return out.astype(np.float32)


def kernel(hidden_states, attention_mask, Wq, bq, Wk, bk, Wv, bv, Wo, bo):
    mask = np.asarray(attention_mask)
    bq_np = np.asarray(bq, dtype=np.float32)
    if (mask == 0).any() or np.any(bq_np):
        # general (never hit with the reference setup_inputs): bq shifts
        # scores per-key and a masked key changes the softmax support —
        # neither is representable in the fast path's fused layout.
        return _numpy_reference(hidden_states, attention_mask, Wq, bq, Wk,
                                bk, Wv, bv, Wo, bo)

    in_maps = _prepare_in_maps(hidden_states, Wq, Wk, Wv, Wo)
    res = _run_device(in_maps)

    # bk only adds a per-query constant to scores (softmax-invariant).
    # bv passes through the probs (rows sum to 1): out += bv @ Wo.T. bo adds.
    extra = (np.asarray(bv, np.float64) @ np.asarray(Wo, np.float64).T
             + np.asarray(bo, np.float64))
    out = np.empty((B, S, D), dtype=np.float32)
    for b in range(B):
        acc = np.zeros((S, D), dtype=np.float64)
        for g in range(GROUPS):
            acc += res.results[b * GROUPS + g]["out"]
        out[b] = (acc + extra).astype(np.float32)
    return out
